# revision 28
# baseline (speedup 1.0000x reference)
"""Trainium2 Bass kernel for nn_Conv1D_MEO (MoE ties-merging + SVD/Kron low-rank).

Strategy (8 NeuronCores, output-channel sharded):
  * Host: SVD top-8 left singular vectors (jax-CPU, matches reference LAPACK
    signs), Kron curve factors -> per-expert rank-8 factors A,B; gating /
    softmax / loss / bias path (all tiny).
  * Device, core c owns OUT rows [128c, 128c+128):
      - ties-merging sign masks (elementwise, DVE+ACT),
      - rank-8 delta reconstruction R = B^T A^T (PE),
      - per-token-group weight combine W_g = res + sum_e g_ge*(mask_e*R_e)
        as K=9 PE matmuls, diagonally tile_position-packed 4x,
      - main matmul y_g^T = W_g^T-slice @ x_g^T (PE, K=1024 accumulation).
  * Host gathers [o,t]-transposed per-core outputs into [B, L, OUT].
"""

import numpy as np

E = 8
IN = 1024
OUT = 1024
B_DIM = 4
L = 2048
T = 256
N = L // T            # 8 groups per batch
NG = B_DIM * N        # 32 token groups
RANK = 8
NCORES = 8
OS = OUT // NCORES    # 128 out channels per core
P = 128
IT = IN // P          # 8 k-tiles
HIT = IT // 2         # k-tiles per half
LOSS_COEF = 1e-3

_PROG = None  # cached (nc, input names)
MM_DT = "f16"   # matmul data dtype: "f16" | "f32r" | "f32" (masks/accum always f32)


def _ensure_paths():
    import sys
    for p in ("/opt/trn_rl_repo", "/opt/pypackages"):
        if p not in sys.path:
            sys.path.append(p)


# ---------------------------------------------------------------- host math

def _softmax_f32(x):
    m = x.max(axis=1, keepdims=True)
    e = np.exp((x - m).astype(np.float32))
    return (e / e.sum(axis=1, keepdims=True)).astype(np.float32)


def _cv_squared(v):
    v = np.asarray(v, np.float32)
    if v.shape[0] == 1:
        return np.float32(0.0)
    return np.float32(np.var(v, ddof=1) / (np.mean(v) ** 2 + 1e-10))


def _host_prep(x, w_gate, weight, bias, res_weight, res_bias,
               curve1_in, curve2_in, curve1_out, curve2_out,
               curve1_bias, curve2_bias):
    """All small/host-side math. Returns per-core input maps + loss."""
    import jax
    import jax.numpy as jnp
    cpu = jax.devices("cpu")[0]

    f32 = np.float32
    x = np.asarray(x, f32)
    diff_w = (np.asarray(weight, f32) - np.asarray(res_weight, f32)[None])

    # --- SVD on jax-CPU: must match the reference's LAPACK sign conventions.
    with jax.default_device(cpu):
        U = np.asarray(jnp.linalg.svd(jnp.asarray(diff_w),
                                      full_matrices=False)[0][:, :, :RANK])
        # sign of the expert-summed weight delta, computed with the exact
        # same reduction the reference uses (fp32 sum order matters on
        # near-zero elements).
        sg_full = np.asarray(jnp.sign(jnp.sum(jnp.asarray(diff_w), axis=0)))
        diff_b = (np.asarray(bias, f32) - np.asarray(res_bias, f32))
        sb = np.asarray(jnp.sign(jnp.asarray(diff_b)))
        ssb = np.asarray(jnp.sign(jnp.sum(jnp.asarray(diff_b), axis=0)))

    # --- low-rank + Kron-factorized reconstruction (tiny matrices)
    Pm = np.swapaxes(U, 1, 2)                       # [E, r, OUT]
    rtw = np.matmul(Pm, diff_w)                     # [E, r, IN]
    rtw = rtw.reshape(E, 2, 4, 32, 32)
    c1o = np.asarray(curve1_out, f32)
    c2o = np.asarray(curve2_out, f32)
    c1i = np.asarray(curve1_in, f32)
    c2i = np.asarray(curve2_in, f32)
    rtw = np.einsum("bij,bjklm->biklm", c1o, rtw).astype(f32)
    rtw = np.einsum("bik,bjklm->bjilm", c2o, rtw).astype(f32)
    rtw = np.einsum("bil,bjklm->bjkim", c1i, rtw).astype(f32)
    rtw = np.einsum("bim,bjklm->bjkli", c2i, rtw).astype(f32)
    Bfac = rtw.reshape(E, RANK, IN)                 # [E, r, IN]
    Ahalf = (U / 6.0).astype(f32)                   # [E, OUT, r]; /6 folds mask u(u^2-1)/6

    # --- gating
    xg = x.reshape(NG, T, IN)
    mx = xg.mean(axis=1, dtype=f32)                 # [NG, IN]
    logits = (mx @ np.asarray(w_gate, f32)).astype(f32)
    gates = _softmax_f32(logits)                    # [NG, E]
    importance = gates.sum(axis=0, dtype=f32)
    load = (gates > 0).sum(axis=0).astype(f32)
    loss = np.float32((_cv_squared(importance) + _cv_squared(load)) * LOSS_COEF)

    e_first = gates.reshape(B_DIM, N, E)[:, 0].copy()
    g2 = np.roll(gates, 1, axis=0).reshape(B_DIM, N, E).copy()
    g2[:, 0] = e_first
    gates_f = np.ascontiguousarray(g2.reshape(NG, E))

    # --- bias path (all tiny)
    mask_b = (sb * (sb == ssb)).astype(f32)
    c1b = np.asarray(curve1_bias, f32)
    c2b = np.asarray(curve2_bias, f32)
    rtb = diff_b.reshape(E, 32, 32)
    rtb = np.einsum("bki,bij->bkj", c1b, rtb).astype(f32)
    rtb = np.einsum("bkj,bij->bik", c2b, rtb).astype(f32)
    rtb = rtb.reshape(E, OUT)
    expert_bias = (np.asarray(res_bias, f32)
                   + gates_f @ (rtb * mask_b)).astype(f32)   # [NG, OUT]

    # --- gate matrix for the device combine: 4 row-group replicas of
    # [gates^T ; ones] padded to 32 rows each.
    gmat = np.zeros((P, NG), f32)
    for rg in range(4):
        gmat[32 * rg:32 * rg + E] = gates_f.T
        gmat[32 * rg + E] = 1.0

    # --- per-core device inputs
    mmnp = {"f16": np.float16}.get(MM_DT, np.float32)
    # B/A rank factors packed across 128 partitions: expert e sits at
    # partitions 32*(e%4)+r, free slot e//4  (row-group packing for the PE).
    Bpack = np.zeros((P, 2, IN), f32)
    Apack = np.zeros((P, 2, OUT), f32)
    for e in range(E):
        Bpack[32 * (e % 4):32 * (e % 4) + RANK, e // 4] = Bfac[e]
        Apack[32 * (e % 4):32 * (e % 4) + RANK, e // 4] = \
            Ahalf[e].T                                       # [r, OUT]
    # Partition permutation: device partition p' holds original i-row
    # inv_perm[p'] within each 128-block, chosen so that the DRAM-scratch
    # restaging/drain DMAs have <=3 strided dims.
    ip = np.array([16 * ((pp % 32) // 4) + 4 * (pp // 32) + pp % 4
                   for pp in range(P)])
    rowperm = np.concatenate([blk * P + ip for blk in range(IN // P)])

    xT = np.ascontiguousarray(x.reshape(NG * T, IN).T[rowperm].astype(mmnp))
    in_maps = []
    for c in range(NCORES):
        osl = slice(OS * c, OS * (c + 1))
        in_maps.append({
            "xT": xT,
            "wT": np.ascontiguousarray(
                np.asarray(weight, f32)[:, osl, :].transpose(0, 2, 1)[:, rowperm]),
            "rT": np.ascontiguousarray(
                np.asarray(res_weight, f32)[osl, :].T[rowperm]),
            "sgT": np.ascontiguousarray(sg_full[osl, :].T[rowperm].astype(f32)),
            "Bf": np.ascontiguousarray(Bpack[:, :, rowperm].astype(mmnp)),
            "Ah": np.ascontiguousarray(Apack[:, :, osl].astype(mmnp)),
            "gm": gmat.astype(mmnp),
            "rTr": np.ascontiguousarray(
                np.asarray(res_weight, f32)[osl, :].T[rowperm].astype(mmnp)),
            "bT": np.ascontiguousarray(expert_bias[:, osl].T),       # [OS, NG]
        })
    return in_maps, loss


# ---------------------------------------------------------------- device kernel

def _build_program():
    _ensure_paths()
    import concourse.bass as bass
    import concourse.mybir as mybir
    import concourse.tile as tile
    from concourse import bacc
    from concourse.bass import ds, ts

    f32 = mybir.dt.float32
    fmm = {"f32r": mybir.dt.float32r,
           "f16": mybir.dt.float16}.get(MM_DT, f32)
    AF = mybir.ActivationFunctionType
    OP = mybir.AluOpType

    nc = bacc.Bacc("TRN2", target_bir_lowering=False, debug=False,
                   num_devices=NCORES)
    xT = nc.dram_tensor("xT", [IN, NG * T], fmm, kind="ExternalInput").ap()
    wT = nc.dram_tensor("wT", [E, IN, OS], f32, kind="ExternalInput").ap()
    rT = nc.dram_tensor("rT", [IN, OS], f32, kind="ExternalInput").ap()
    sgT = nc.dram_tensor("sgT", [IN, OS], f32, kind="ExternalInput").ap()
    Bf = nc.dram_tensor("Bf", [P, 2, IN], fmm, kind="ExternalInput").ap()
    Ah = nc.dram_tensor("Ah", [P, 2, OS], fmm, kind="ExternalInput").ap()
    gm = nc.dram_tensor("gm", [P, NG], fmm, kind="ExternalInput").ap()
    rTr = nc.dram_tensor("rTr", [IN, OS], fmm, kind="ExternalInput").ap()
    bT = nc.dram_tensor("bT", [OS, NG], f32, kind="ExternalInput").ap()
    yO = nc.dram_tensor("y", [NG, OS, T], f32, kind="ExternalOutput").ap()
    Dscr = nc.dram_tensor("Dscr", [IT // 2, P, E + 1, 2, OS], fmm).ap()
    Wscr = nc.dram_tensor("Wscr", [NG, IN, OS], fmm).ap()

    QIT = 2                       # k-tiles per quarter
    NQ = IT // QIT                # 4 quarters

    with tile.TileContext(nc) as tc:
        from contextlib import ExitStack
        with ExitStack() as ctx:
            const = ctx.enter_context(tc.tile_pool(name="const", bufs=1))
            wpool = ctx.enter_context(tc.tile_pool(name="wpool", bufs=3))
            dpool = ctx.enter_context(tc.tile_pool(name="dpool", bufs=3))
            Wp = ctx.enter_context(tc.tile_pool(name="Wp", bufs=5))
            ypool = ctx.enter_context(tc.tile_pool(name="ypool", bufs=1))
            mp = ctx.enter_context(tc.tile_pool(name="mp", bufs=4))
            rq = ctx.enter_context(tc.tile_pool(name="rq", bufs=1))
            sp = ctx.enter_context(tc.tile_pool(name="sp", bufs=3))
            cb = ctx.enter_context(tc.tile_pool(name="cb", bufs=3))
            xp = ctx.enter_context(tc.tile_pool(name="xp", bufs=8))
            pr_p = ctx.enter_context(tc.tile_pool(name="pr", bufs=2, space="PSUM"))
            pc_p = ctx.enter_context(tc.tile_pool(name="pc", bufs=4, space="PSUM"))
            py_p = ctx.enter_context(tc.tile_pool(name="py", bufs=2, space="PSUM"))

            B_sb = const.tile([P, 2, IN], fmm)
            nc.sync.dma_start(B_sb[:], Bf)
            A_sb = const.tile([P, 2, OS], fmm)
            nc.sync.dma_start(A_sb[:], Ah)
            g_sb = const.tile([P, NG], fmm)
            nc.sync.dma_start(g_sb[:], gm)
            b_sb = const.tile([OS, NG], f32)
            nc.sync.dma_start(b_sb[:], bT)
            r_sb = const.tile([P, IT, OS], f32)
            nc.sync.dma_start(r_sb[:], rT.rearrange("(it p) o -> p it o", p=P))
            s_sb = const.tile([P, IT, OS], f32)
            nc.sync.dma_start(s_sb[:], sgT.rearrange("(it p) o -> p it o", p=P))

            yac = ypool.tile([OS, NG, T], f32)

            for qh in range(NQ):
                it0 = qh * QIT

                # ---- per-quarter weight load + masks + rank-8 delta
                # Dt [ip, e9, jt, o]; row e9==E holds the residual weights.
                wt = wpool.tile([P, E, QIT, OS], f32, tag="wt")
                for e in range(E):
                    nc.sync.dma_start(
                        wt[:, e],
                        wT[e, ds(it0 * P, QIT * P), :].rearrange(
                            "(it p) o -> p it o", p=P))
                Dt = dpool.tile([P, E + 1, QIT, OS], fmm, tag="Dt")
                nc.scalar.dma_start(
                    Dt[:, E],
                    rTr[ds(it0 * P, QIT * P), :].rearrange(
                        "(it p) o -> p it o", p=P))
                for e in range(E):
                    d = mp.tile([P, QIT, OS], f32, tag="t1")
                    nc.gpsimd.tensor_tensor(d[:], wt[:, e],
                                            r_sb[:, ds(it0, QIT)],
                                            OP.subtract)
                    sd = mp.tile([P, QIT, OS], f32, tag="t2")
                    nc.scalar.activation(sd[:], d[:], AF.Sign)
                    u = mp.tile([P, QIT, OS], f32, tag="t5")
                    nc.gpsimd.tensor_tensor(u[:], sd[:],
                                            s_sb[:, ds(it0, QIT)], OP.add)
                    sq = mp.tile([P, QIT, OS], f32, tag="t6")
                    nc.gpsimd.tensor_tensor(sq[:], u[:], u[:], OP.mult)
                    v = mp.tile([P, QIT, OS], f32, tag="t4")
                    nc.vector.tensor_scalar_add(v[:], sq[:], -1.0)
                    um = mp.tile([P, QIT, OS], f32, tag="t3")
                    nc.vector.tensor_tensor(um[:], u[:], v[:], OP.mult)
                    for j in range(QIT):
                        it = it0 + j
                        prt = pr_p.tile([P, OS], f32, tag="pr")
                        nc.tensor.matmul(prt[:],
                                         B_sb[ds(32 * (e % 4), RANK),
                                              e // 4, ts(it, P)],
                                         A_sb[ds(32 * (e % 4), RANK),
                                              e // 4, :],
                                         start=True, stop=True,
                                         tile_position=(32 * (e % 4), 0))
                        nc.vector.tensor_tensor(Dt[:, e, j], um[:, j],
                                                prt[:], OP.mult)

                # ---- combine + drain into W quarter
                # Partition p' of Dt holds i_loc = 16*(p'%32//4) + 4*(p'//32)
                # + p'%4 (host-permuted), so the e-partition restaging and
                # the W drain are each a handful of big <=3-dim DMAs through
                # DRAM scratch. MM q = 4*G+rg covers rhs slots 4G..4G+3 of
                # row-block rg (slot sl = p' - 32*rg).
                nc.gpsimd.dma_start(Dscr[qh], Dt[:])
                dma_rot = [nc.sync, nc.scalar, nc.gpsimd]
                Whs = []
                for j in range(QIT):
                    rhs = sp.tile([P, 32, OS], fmm, tag="rhs")
                    for rg in range(4):
                        nc.sync.dma_start(
                            rhs[ds(32 * rg, E + 1), :, :],
                            bass.AP(tensor=Dscr.tensor,
                                    offset=qh * (P * (E + 1) * QIT * OS)
                                    + 32 * rg * ((E + 1) * QIT * OS)
                                    + j * OS,
                                    ap=[[QIT * OS, E + 1],
                                        [(E + 1) * QIT * OS, 32],
                                        [1, OS]]))
                    for G in range(8):
                        ps4 = pc_p.tile([P, 4, OS], f32, tag="pc")
                        for rg in range(4):
                            nc.tensor.matmul(
                                ps4[ds(32 * rg, NG), :, :],
                                g_sb[ds(32 * rg, E + 1), :],
                                rhs[ds(32 * rg, E + 1), ds(4 * G, 4), :],
                                start=True, stop=True,
                                tile_position=(32 * rg, 32 * rg))
                        cbuf = cb.tile([P, 4, OS], fmm, tag="cb")
                        if G % 4 == 3:
                            nc.vector.tensor_copy(cbuf[:], ps4[:])
                        else:
                            nc.scalar.activation(cbuf[:], ps4[:], AF.Copy)
                        dma_rot[G % 3].dma_start(
                            bass.AP(tensor=Wscr.tensor,
                                    offset=((it0 + j) * P + 4 * G) * OS,
                                    ap=[[32 * OS, 4], [IN * OS, NG],
                                        [1, 4 * OS]]),
                            cbuf[:])
                    # ---- W readback for this j
                    Wh = Wp.tile([P, NG, OS], fmm, tag="W")
                    nc.gpsimd.dma_start(
                        Wh[:],
                        Wscr[:, ds((it0 + j) * P, P), :].rearrange(
                            "g i o -> i g o"))
                    Whs.append(Wh)

                # ---- main matmul for this quarter (chained over j in PSUM)
                for gp in range(NG // 2):
                    xts = []
                    for j in range(QIT):
                        it = it0 + j
                        xt = xp.tile([P, 2 * T], fmm, tag="xt")
                        eng = nc.sync if (gp + j) % 2 == 0 else nc.scalar
                        eng.dma_start(xt[:], xT[ds(it * P, P),
                                                ds(gp * 2 * T, 2 * T)])
                        xts.append(xt)
                    for gl in range(2):
                        g = 2 * gp + gl
                        pyt = py_p.tile([P, T], f32, tag="py")
                        for j in range(QIT):
                            nc.tensor.matmul(pyt[:], Whs[j][:, g, :],
                                             xts[j][:, ds(gl * T, T)],
                                             start=(j == 0),
                                             stop=(j == QIT - 1))
                        if qh == 0:
                            nc.scalar.activation(yac[:, g, :], pyt[:],
                                                 AF.Identity,
                                                 bias=b_sb[:, ds(g, 1)])
                        else:
                            nc.vector.tensor_tensor(yac[:, g, :],
                                                    yac[:, g, :],
                                                    pyt[:], OP.add)

            nc.sync.dma_start(yO.rearrange("g o t -> o g t"), yac[:])

    nc.compile()
    return nc


def _get_program():
    global _PROG
    if _PROG is None:
        _PROG = _build_program()
    return _PROG


# ---------------------------------------------------------------- entry point

def _run(in_maps, trace=False):
    _ensure_paths()
    from concourse.bass_utils import run_bass_kernel_spmd
    nc = _get_program()
    return run_bass_kernel_spmd(nc, in_maps, core_ids=list(range(NCORES)),
                                trace=trace)


def kernel(**inputs):
    _ensure_paths()
    in_maps, loss = _host_prep(**inputs)
    res = _run(in_maps, trace=False)
    ys = [res.results[c]["y"] for c in range(NCORES)]
    Y = np.stack(ys)                                   # [C, NG, OS, T]
    y = np.transpose(Y, (1, 3, 0, 2)).reshape(NG, T, OUT)
    y = np.ascontiguousarray(y.reshape(B_DIM, L, OUT), dtype=np.float32)
    return y, loss


# revision 29
# speedup vs baseline: 1.0259x; 1.0259x over previous
"""Trainium2 Bass kernel for nn_Conv1D_MEO (MoE ties-merging + SVD/Kron low-rank).

Strategy (8 NeuronCores, output-channel sharded):
  * Host: SVD top-8 left singular vectors (jax-CPU, matches reference LAPACK
    signs), Kron curve factors -> per-expert rank-8 factors A,B; gating /
    softmax / loss / bias path (all tiny).
  * Device, core c owns OUT rows [128c, 128c+128):
      - ties-merging sign masks (elementwise, DVE+ACT),
      - rank-8 delta reconstruction R = B^T A^T (PE),
      - per-token-group weight combine W_g = res + sum_e g_ge*(mask_e*R_e)
        as K=9 PE matmuls, diagonally tile_position-packed 4x,
      - main matmul y_g^T = W_g^T-slice @ x_g^T (PE, K=1024 accumulation).
  * Host gathers [o,t]-transposed per-core outputs into [B, L, OUT].
"""

import numpy as np

E = 8
IN = 1024
OUT = 1024
B_DIM = 4
L = 2048
T = 256
N = L // T            # 8 groups per batch
NG = B_DIM * N        # 32 token groups
RANK = 8
NCORES = 8
OS = OUT // NCORES    # 128 out channels per core
P = 128
IT = IN // P          # 8 k-tiles
HIT = IT // 2         # k-tiles per half
LOSS_COEF = 1e-3

_PROG = None  # cached (nc, input names)
MM_DT = "f16"   # matmul data dtype: "f16" | "f32r" | "f32" (masks/accum always f32)


def _ensure_paths():
    import sys
    for p in ("/opt/trn_rl_repo", "/opt/pypackages"):
        if p not in sys.path:
            sys.path.append(p)


# ---------------------------------------------------------------- host math

def _softmax_f32(x):
    m = x.max(axis=1, keepdims=True)
    e = np.exp((x - m).astype(np.float32))
    return (e / e.sum(axis=1, keepdims=True)).astype(np.float32)


def _cv_squared(v):
    v = np.asarray(v, np.float32)
    if v.shape[0] == 1:
        return np.float32(0.0)
    return np.float32(np.var(v, ddof=1) / (np.mean(v) ** 2 + 1e-10))


def _host_prep(x, w_gate, weight, bias, res_weight, res_bias,
               curve1_in, curve2_in, curve1_out, curve2_out,
               curve1_bias, curve2_bias):
    """All small/host-side math. Returns per-core input maps + loss."""
    import jax
    import jax.numpy as jnp
    cpu = jax.devices("cpu")[0]

    f32 = np.float32
    x = np.asarray(x, f32)
    diff_w = (np.asarray(weight, f32) - np.asarray(res_weight, f32)[None])

    # --- SVD on jax-CPU: must match the reference's LAPACK sign conventions.
    with jax.default_device(cpu):
        U = np.asarray(jnp.linalg.svd(jnp.asarray(diff_w),
                                      full_matrices=False)[0][:, :, :RANK])
        # sign of the expert-summed weight delta, computed with the exact
        # same reduction the reference uses (fp32 sum order matters on
        # near-zero elements).
        sg_full = np.asarray(jnp.sign(jnp.sum(jnp.asarray(diff_w), axis=0)))
        diff_b = (np.asarray(bias, f32) - np.asarray(res_bias, f32))
        sb = np.asarray(jnp.sign(jnp.asarray(diff_b)))
        ssb = np.asarray(jnp.sign(jnp.sum(jnp.asarray(diff_b), axis=0)))

    # --- low-rank + Kron-factorized reconstruction (tiny matrices)
    Pm = np.swapaxes(U, 1, 2)                       # [E, r, OUT]
    rtw = np.matmul(Pm, diff_w)                     # [E, r, IN]
    rtw = rtw.reshape(E, 2, 4, 32, 32)
    c1o = np.asarray(curve1_out, f32)
    c2o = np.asarray(curve2_out, f32)
    c1i = np.asarray(curve1_in, f32)
    c2i = np.asarray(curve2_in, f32)
    rtw = np.einsum("bij,bjklm->biklm", c1o, rtw).astype(f32)
    rtw = np.einsum("bik,bjklm->bjilm", c2o, rtw).astype(f32)
    rtw = np.einsum("bil,bjklm->bjkim", c1i, rtw).astype(f32)
    rtw = np.einsum("bim,bjklm->bjkli", c2i, rtw).astype(f32)
    Bfac = rtw.reshape(E, RANK, IN)                 # [E, r, IN]
    Ahalf = (U / 6.0).astype(f32)                   # [E, OUT, r]; /6 folds mask u(u^2-1)/6

    # --- gating
    xg = x.reshape(NG, T, IN)
    mx = xg.mean(axis=1, dtype=f32)                 # [NG, IN]
    logits = (mx @ np.asarray(w_gate, f32)).astype(f32)
    gates = _softmax_f32(logits)                    # [NG, E]
    importance = gates.sum(axis=0, dtype=f32)
    load = (gates > 0).sum(axis=0).astype(f32)
    loss = np.float32((_cv_squared(importance) + _cv_squared(load)) * LOSS_COEF)

    e_first = gates.reshape(B_DIM, N, E)[:, 0].copy()
    g2 = np.roll(gates, 1, axis=0).reshape(B_DIM, N, E).copy()
    g2[:, 0] = e_first
    gates_f = np.ascontiguousarray(g2.reshape(NG, E))

    # --- bias path (all tiny)
    mask_b = (sb * (sb == ssb)).astype(f32)
    c1b = np.asarray(curve1_bias, f32)
    c2b = np.asarray(curve2_bias, f32)
    rtb = diff_b.reshape(E, 32, 32)
    rtb = np.einsum("bki,bij->bkj", c1b, rtb).astype(f32)
    rtb = np.einsum("bkj,bij->bik", c2b, rtb).astype(f32)
    rtb = rtb.reshape(E, OUT)
    expert_bias = (np.asarray(res_bias, f32)
                   + gates_f @ (rtb * mask_b)).astype(f32)   # [NG, OUT]

    # --- gate matrix for the device combine: 4 row-group replicas of
    # [gates^T ; ones] padded to 32 rows each.
    gmat = np.zeros((P, NG), f32)
    for rg in range(4):
        gmat[32 * rg:32 * rg + E] = gates_f.T
        gmat[32 * rg + E] = 1.0

    # --- per-core device inputs
    mmnp = {"f16": np.float16}.get(MM_DT, np.float32)
    # B/A rank factors packed across 128 partitions: expert e sits at
    # partitions 32*(e%4)+r, free slot e//4  (row-group packing for the PE).
    Bpack = np.zeros((P, 2, IN), f32)
    Apack = np.zeros((P, 2, OUT), f32)
    for e in range(E):
        Bpack[32 * (e % 4):32 * (e % 4) + RANK, e // 4] = Bfac[e]
        Apack[32 * (e % 4):32 * (e % 4) + RANK, e // 4] = \
            Ahalf[e].T                                       # [r, OUT]
    # Partition permutation: device partition p' holds original i-row
    # inv_perm[p'] within each 128-block, chosen so that the DRAM-scratch
    # restaging/drain DMAs have <=3 strided dims.
    ip = np.array([16 * ((pp % 32) // 4) + 4 * (pp // 32) + pp % 4
                   for pp in range(P)])
    rowperm = np.concatenate([blk * P + ip for blk in range(IN // P)])

    xT = np.ascontiguousarray(x.reshape(NG * T, IN).T[rowperm].astype(mmnp))
    in_maps = []
    for c in range(NCORES):
        osl = slice(OS * c, OS * (c + 1))
        in_maps.append({
            "xT": xT,
            "wT": np.ascontiguousarray(
                np.asarray(weight, f32)[:, osl, :].transpose(0, 2, 1)[:, rowperm]),
            "rT": np.ascontiguousarray(
                np.asarray(res_weight, f32)[osl, :].T[rowperm]),
            "sgT": np.ascontiguousarray(sg_full[osl, :].T[rowperm].astype(f32)),
            "Bf": np.ascontiguousarray(Bpack[:, :, rowperm].astype(mmnp)),
            "Ah": np.ascontiguousarray(Apack[:, :, osl].astype(mmnp)),
            "gm": gmat.astype(mmnp),
            "rTr": np.ascontiguousarray(
                np.asarray(res_weight, f32)[osl, :].T[rowperm].astype(mmnp)),
            "bT": np.ascontiguousarray(expert_bias[:, osl].T),       # [OS, NG]
        })
    return in_maps, loss


# ---------------------------------------------------------------- device kernel

def _build_program():
    _ensure_paths()
    import concourse.bass as bass
    import concourse.mybir as mybir
    import concourse.tile as tile
    from concourse import bacc
    from concourse.bass import ds, ts

    f32 = mybir.dt.float32
    fmm = {"f32r": mybir.dt.float32r,
           "f16": mybir.dt.float16}.get(MM_DT, f32)
    AF = mybir.ActivationFunctionType
    OP = mybir.AluOpType

    nc = bacc.Bacc("TRN2", target_bir_lowering=False, debug=False,
                   num_devices=NCORES)
    xT = nc.dram_tensor("xT", [IN, NG * T], fmm, kind="ExternalInput").ap()
    wT = nc.dram_tensor("wT", [E, IN, OS], f32, kind="ExternalInput").ap()
    rT = nc.dram_tensor("rT", [IN, OS], f32, kind="ExternalInput").ap()
    sgT = nc.dram_tensor("sgT", [IN, OS], f32, kind="ExternalInput").ap()
    Bf = nc.dram_tensor("Bf", [P, 2, IN], fmm, kind="ExternalInput").ap()
    Ah = nc.dram_tensor("Ah", [P, 2, OS], fmm, kind="ExternalInput").ap()
    gm = nc.dram_tensor("gm", [P, NG], fmm, kind="ExternalInput").ap()
    rTr = nc.dram_tensor("rTr", [IN, OS], fmm, kind="ExternalInput").ap()
    bT = nc.dram_tensor("bT", [OS, NG], f32, kind="ExternalInput").ap()
    yO = nc.dram_tensor("y", [NG, OS, T], f32, kind="ExternalOutput").ap()
    Dscr = nc.dram_tensor("Dscr", [IT // 2, P, E + 1, 2, OS], fmm).ap()
    Wscr = nc.dram_tensor("Wscr", [NG, IN, OS], fmm).ap()

    QIT = 2                       # k-tiles per quarter
    NQ = IT // QIT                # 4 quarters

    with tile.TileContext(nc) as tc:
        from contextlib import ExitStack
        with ExitStack() as ctx:
            const = ctx.enter_context(tc.tile_pool(name="const", bufs=1))
            wpool = ctx.enter_context(tc.tile_pool(name="wpool", bufs=2))
            dpool = ctx.enter_context(tc.tile_pool(name="dpool", bufs=2))
            Wp = ctx.enter_context(tc.tile_pool(name="Wp", bufs=5))
            ypool = ctx.enter_context(tc.tile_pool(name="ypool", bufs=1))
            mp = ctx.enter_context(tc.tile_pool(name="mp", bufs=4))
            rq = ctx.enter_context(tc.tile_pool(name="rq", bufs=1))
            sp = ctx.enter_context(tc.tile_pool(name="sp", bufs=3))
            cb = ctx.enter_context(tc.tile_pool(name="cb", bufs=3))
            xp = ctx.enter_context(tc.tile_pool(name="xp", bufs=8))
            pr_p = ctx.enter_context(tc.tile_pool(name="pr", bufs=2, space="PSUM"))
            pc_p = ctx.enter_context(tc.tile_pool(name="pc", bufs=3, space="PSUM"))
            py_p = ctx.enter_context(tc.tile_pool(name="py", bufs=3, space="PSUM"))

            B_sb = const.tile([P, 2, IN], fmm)
            nc.sync.dma_start(B_sb[:], Bf)
            A_sb = const.tile([P, 2, OS], fmm)
            nc.sync.dma_start(A_sb[:], Ah)
            g_sb = const.tile([P, NG], fmm)
            nc.sync.dma_start(g_sb[:], gm)
            b_sb = const.tile([OS, NG], f32)
            nc.sync.dma_start(b_sb[:], bT)
            r_sb = const.tile([P, IT, OS], f32)
            nc.sync.dma_start(r_sb[:], rT.rearrange("(it p) o -> p it o", p=P))
            s_sb = const.tile([P, IT, OS], f32)
            nc.sync.dma_start(s_sb[:], sgT.rearrange("(it p) o -> p it o", p=P))

            yac = ypool.tile([OS, NG, T], f32)

            for qh in range(NQ):
                it0 = qh * QIT

                # ---- per-quarter weight load + masks + rank-8 delta
                # Dt [ip, e9, jt, o]; row e9==E holds the residual weights.
                wt = wpool.tile([P, E, QIT, OS], f32, tag="wt")
                for e in range(E):
                    nc.sync.dma_start(
                        wt[:, e],
                        wT[e, ds(it0 * P, QIT * P), :].rearrange(
                            "(it p) o -> p it o", p=P))
                Dt = dpool.tile([P, E + 1, QIT, OS], fmm, tag="Dt")
                nc.scalar.dma_start(
                    Dt[:, E],
                    rTr[ds(it0 * P, QIT * P), :].rearrange(
                        "(it p) o -> p it o", p=P))
                for e in range(E):
                    d = mp.tile([P, QIT, OS], f32, tag="t1")
                    nc.gpsimd.tensor_tensor(d[:], wt[:, e],
                                            r_sb[:, ds(it0, QIT)],
                                            OP.subtract)
                    sd = mp.tile([P, QIT, OS], f32, tag="t2")
                    nc.scalar.activation(sd[:], d[:], AF.Sign)
                    u = mp.tile([P, QIT, OS], f32, tag="t5")
                    nc.gpsimd.tensor_tensor(u[:], sd[:],
                                            s_sb[:, ds(it0, QIT)], OP.add)
                    sq = mp.tile([P, QIT, OS], f32, tag="t6")
                    nc.gpsimd.tensor_tensor(sq[:], u[:], u[:], OP.mult)
                    v = mp.tile([P, QIT, OS], f32, tag="t4")
                    nc.vector.tensor_scalar_add(v[:], sq[:], -1.0)
                    um = mp.tile([P, QIT, OS], f32, tag="t3")
                    nc.vector.tensor_tensor(um[:], u[:], v[:], OP.mult)
                    for j in range(QIT):
                        it = it0 + j
                        prt = pr_p.tile([P, OS], f32, tag="pr")
                        nc.tensor.matmul(prt[:],
                                         B_sb[ds(32 * (e % 4), RANK),
                                              e // 4, ts(it, P)],
                                         A_sb[ds(32 * (e % 4), RANK),
                                              e // 4, :],
                                         start=True, stop=True,
                                         tile_position=(32 * (e % 4), 0))
                        nc.vector.tensor_tensor(Dt[:, e, j], um[:, j],
                                                prt[:], OP.mult)

                # ---- combine + drain into W quarter
                # Partition p' of Dt holds i_loc = 16*(p'%32//4) + 4*(p'//32)
                # + p'%4 (host-permuted), so the e-partition restaging and
                # the W drain are each a handful of big <=3-dim DMAs through
                # DRAM scratch. MM q = 4*G+rg covers rhs slots 4G..4G+3 of
                # row-block rg (slot sl = p' - 32*rg).
                nc.gpsimd.dma_start(Dscr[qh], Dt[:])
                dma_rot = [nc.sync, nc.scalar, nc.gpsimd]
                Whs = []
                for j in range(QIT):
                    rhs = sp.tile([P, 32, OS], fmm, tag="rhs")
                    for rg in range(4):
                        nc.sync.dma_start(
                            rhs[ds(32 * rg, E + 1), :, :],
                            bass.AP(tensor=Dscr.tensor,
                                    offset=qh * (P * (E + 1) * QIT * OS)
                                    + 32 * rg * ((E + 1) * QIT * OS)
                                    + j * OS,
                                    ap=[[QIT * OS, E + 1],
                                        [(E + 1) * QIT * OS, 32],
                                        [1, OS]]))
                    for G in range(8):
                        ps4 = pc_p.tile([P, 4, OS], f32, tag="pc")
                        for rg in range(4):
                            nc.tensor.matmul(
                                ps4[ds(32 * rg, NG), :, :],
                                g_sb[ds(32 * rg, E + 1), :],
                                rhs[ds(32 * rg, E + 1), ds(4 * G, 4), :],
                                start=True, stop=True,
                                tile_position=(32 * rg, 32 * rg))
                        cbuf = cb.tile([P, 4, OS], fmm, tag="cb")
                        if G % 2 == 0:
                            nc.vector.tensor_copy(cbuf[:], ps4[:])
                        else:
                            nc.scalar.activation(cbuf[:], ps4[:], AF.Copy)
                        dma_rot[G % 3].dma_start(
                            bass.AP(tensor=Wscr.tensor,
                                    offset=((it0 + j) * P + 4 * G) * OS,
                                    ap=[[32 * OS, 4], [IN * OS, NG],
                                        [1, 4 * OS]]),
                            cbuf[:])
                    # ---- W readback for this j
                    Wh = Wp.tile([P, NG, OS], fmm, tag="W")
                    nc.scalar.dma_start(
                        Wh[:],
                        Wscr[:, ds((it0 + j) * P, P), :].rearrange(
                            "g i o -> i g o"))
                    Whs.append(Wh)

                # ---- main matmul for this quarter (chained over j in PSUM)
                for gp in range(NG // 2):
                    xts = []
                    for j in range(QIT):
                        it = it0 + j
                        xt = xp.tile([P, 2 * T], fmm, tag="xt")
                        eng = nc.sync if (gp + j) % 2 == 0 else nc.scalar
                        eng.dma_start(xt[:], xT[ds(it * P, P),
                                                ds(gp * 2 * T, 2 * T)])
                        xts.append(xt)
                    for gl in range(2):
                        g = 2 * gp + gl
                        pyt = py_p.tile([P, T], f32, tag="py")
                        for j in range(QIT):
                            nc.tensor.matmul(pyt[:], Whs[j][:, g, :],
                                             xts[j][:, ds(gl * T, T)],
                                             start=(j == 0),
                                             stop=(j == QIT - 1))
                        if qh == 0:
                            nc.scalar.activation(yac[:, g, :], pyt[:],
                                                 AF.Identity,
                                                 bias=b_sb[:, ds(g, 1)])
                        else:
                            nc.vector.tensor_tensor(yac[:, g, :],
                                                    yac[:, g, :],
                                                    pyt[:], OP.add)

            nc.sync.dma_start(yO.rearrange("g o t -> o g t"), yac[:])

    nc.compile()
    return nc


def _get_program():
    global _PROG
    if _PROG is None:
        _PROG = _build_program()
    return _PROG


# ---------------------------------------------------------------- entry point

def _run(in_maps, trace=False):
    _ensure_paths()
    from concourse.bass_utils import run_bass_kernel_spmd
    nc = _get_program()
    return run_bass_kernel_spmd(nc, in_maps, core_ids=list(range(NCORES)),
                                trace=trace)


def kernel(**inputs):
    _ensure_paths()
    in_maps, loss = _host_prep(**inputs)
    res = _run(in_maps, trace=False)
    ys = [res.results[c]["y"] for c in range(NCORES)]
    Y = np.stack(ys)                                   # [C, NG, OS, T]
    y = np.transpose(Y, (1, 3, 0, 2)).reshape(NG, T, OUT)
    y = np.ascontiguousarray(y.reshape(B_DIM, L, OUT), dtype=np.float32)
    return y, loss


# revision 30
# speedup vs baseline: 1.0839x; 1.0565x over previous
"""Trainium2 Bass kernel for nn_Conv1D_MEO (MoE ties-merging + SVD/Kron low-rank).

Strategy (8 NeuronCores, output-channel sharded):
  * Host: SVD top-8 left singular vectors (jax-CPU, matches reference LAPACK
    signs), Kron curve factors -> per-expert rank-8 factors A,B; gating /
    softmax / loss / bias path (all tiny).
  * Device, core c owns OUT rows [128c, 128c+128):
      - ties-merging sign masks (elementwise, DVE+ACT),
      - rank-8 delta reconstruction R = B^T A^T (PE),
      - per-token-group weight combine W_g = res + sum_e g_ge*(mask_e*R_e)
        as K=9 PE matmuls, diagonally tile_position-packed 4x,
      - main matmul y_g^T = W_g^T-slice @ x_g^T (PE, K=1024 accumulation).
  * Host gathers [o,t]-transposed per-core outputs into [B, L, OUT].
"""

import numpy as np

E = 8
IN = 1024
OUT = 1024
B_DIM = 4
L = 2048
T = 256
N = L // T            # 8 groups per batch
NG = B_DIM * N        # 32 token groups
RANK = 8
NCORES = 8
OS = OUT // NCORES    # 128 out channels per core
P = 128
IT = IN // P          # 8 k-tiles
HIT = IT // 2         # k-tiles per half
LOSS_COEF = 1e-3

_PROG = None  # cached (nc, input names)
MM_DT = "f16"   # matmul data dtype: "f16" | "f32r" | "f32" (masks/accum always f32)


def _ensure_paths():
    import sys
    for p in ("/opt/trn_rl_repo", "/opt/pypackages"):
        if p not in sys.path:
            sys.path.append(p)


# ---------------------------------------------------------------- host math

def _softmax_f32(x):
    m = x.max(axis=1, keepdims=True)
    e = np.exp((x - m).astype(np.float32))
    return (e / e.sum(axis=1, keepdims=True)).astype(np.float32)


def _cv_squared(v):
    v = np.asarray(v, np.float32)
    if v.shape[0] == 1:
        return np.float32(0.0)
    return np.float32(np.var(v, ddof=1) / (np.mean(v) ** 2 + 1e-10))


def _host_prep(x, w_gate, weight, bias, res_weight, res_bias,
               curve1_in, curve2_in, curve1_out, curve2_out,
               curve1_bias, curve2_bias):
    """All small/host-side math. Returns per-core input maps + loss."""
    import jax
    import jax.numpy as jnp
    cpu = jax.devices("cpu")[0]

    f32 = np.float32
    x = np.asarray(x, f32)
    diff_w = (np.asarray(weight, f32) - np.asarray(res_weight, f32)[None])

    # --- SVD on jax-CPU: must match the reference's LAPACK sign conventions.
    with jax.default_device(cpu):
        U = np.asarray(jnp.linalg.svd(jnp.asarray(diff_w),
                                      full_matrices=False)[0][:, :, :RANK])
        # sign of the expert-summed weight delta, computed with the exact
        # same reduction the reference uses (fp32 sum order matters on
        # near-zero elements).
        sg_full = np.asarray(jnp.sign(jnp.sum(jnp.asarray(diff_w), axis=0)))
        diff_b = (np.asarray(bias, f32) - np.asarray(res_bias, f32))
        sb = np.asarray(jnp.sign(jnp.asarray(diff_b)))
        ssb = np.asarray(jnp.sign(jnp.sum(jnp.asarray(diff_b), axis=0)))

    # --- low-rank + Kron-factorized reconstruction (tiny matrices)
    Pm = np.swapaxes(U, 1, 2)                       # [E, r, OUT]
    rtw = np.matmul(Pm, diff_w)                     # [E, r, IN]
    rtw = rtw.reshape(E, 2, 4, 32, 32)
    c1o = np.asarray(curve1_out, f32)
    c2o = np.asarray(curve2_out, f32)
    c1i = np.asarray(curve1_in, f32)
    c2i = np.asarray(curve2_in, f32)
    rtw = np.einsum("bij,bjklm->biklm", c1o, rtw).astype(f32)
    rtw = np.einsum("bik,bjklm->bjilm", c2o, rtw).astype(f32)
    rtw = np.einsum("bil,bjklm->bjkim", c1i, rtw).astype(f32)
    rtw = np.einsum("bim,bjklm->bjkli", c2i, rtw).astype(f32)
    Bfac = rtw.reshape(E, RANK, IN)                 # [E, r, IN]
    Ahalf = U.astype(f32)                           # [E, OUT, r]

    # --- gating
    xg = x.reshape(NG, T, IN)
    mx = xg.mean(axis=1, dtype=f32)                 # [NG, IN]
    logits = (mx @ np.asarray(w_gate, f32)).astype(f32)
    gates = _softmax_f32(logits)                    # [NG, E]
    importance = gates.sum(axis=0, dtype=f32)
    load = (gates > 0).sum(axis=0).astype(f32)
    loss = np.float32((_cv_squared(importance) + _cv_squared(load)) * LOSS_COEF)

    e_first = gates.reshape(B_DIM, N, E)[:, 0].copy()
    g2 = np.roll(gates, 1, axis=0).reshape(B_DIM, N, E).copy()
    g2[:, 0] = e_first
    gates_f = np.ascontiguousarray(g2.reshape(NG, E))

    # --- bias path (all tiny)
    mask_b = (sb * (sb == ssb)).astype(f32)
    c1b = np.asarray(curve1_bias, f32)
    c2b = np.asarray(curve2_bias, f32)
    rtb = diff_b.reshape(E, 32, 32)
    rtb = np.einsum("bki,bij->bkj", c1b, rtb).astype(f32)
    rtb = np.einsum("bkj,bij->bik", c2b, rtb).astype(f32)
    rtb = rtb.reshape(E, OUT)
    expert_bias = (np.asarray(res_bias, f32)
                   + gates_f @ (rtb * mask_b)).astype(f32)   # [NG, OUT]

    # --- gate matrix for the device combine: 4 row-group replicas of
    # [gates^T ; ones] padded to 32 rows each.
    gmat = np.zeros((P, NG), f32)
    for rg in range(4):
        gmat[32 * rg:32 * rg + E] = gates_f.T
        gmat[32 * rg + E] = 1.0

    # --- per-core device inputs
    mmnp = {"f16": np.float16}.get(MM_DT, np.float32)
    # B/A rank factors packed across 128 partitions: expert e sits at
    # partitions 32*(e%4)+r, free slot e//4  (row-group packing for the PE).
    Bpack = np.zeros((P, 2, IN), f32)
    Apack = np.zeros((P, 2, OUT), f32)
    for e in range(E):
        Bpack[32 * (e % 4):32 * (e % 4) + RANK, e // 4] = Bfac[e]
        Apack[32 * (e % 4):32 * (e % 4) + RANK, e // 4] = \
            Ahalf[e].T                                       # [r, OUT]
    # Partition permutation: device partition p' holds original i-row
    # inv_perm[p'] within each 128-block, chosen so that the DRAM-scratch
    # restaging/drain DMAs have <=3 strided dims.
    ip = np.array([16 * ((pp % 32) // 4) + 4 * (pp // 32) + pp % 4
                   for pp in range(P)])
    rowperm = np.concatenate([blk * P + ip for blk in range(IN // P)])

    xT = np.ascontiguousarray(x.reshape(NG * T, IN).T[rowperm].astype(mmnp))
    in_maps = []
    for c in range(NCORES):
        osl = slice(OS * c, OS * (c + 1))
        in_maps.append({
            "xT": xT,
            "wT": np.ascontiguousarray(
                np.asarray(weight, f32)[:, osl, :].transpose(0, 2, 1)[:, rowperm]),
            "rT": np.ascontiguousarray(
                np.asarray(res_weight, f32)[osl, :].T[rowperm]),
            "sgT": np.ascontiguousarray(sg_full[osl, :].T[rowperm].astype(f32)),
            "Bf": np.ascontiguousarray(Bpack[:, :, rowperm].astype(mmnp)),
            "Ah": np.ascontiguousarray(Apack[:, :, osl].astype(mmnp)),
            "gm": gmat.astype(mmnp),
            "rTr": np.ascontiguousarray(
                np.asarray(res_weight, f32)[osl, :].T[rowperm].astype(mmnp)),
            "bT": np.ascontiguousarray(expert_bias[:, osl].T),       # [OS, NG]
        })
    return in_maps, loss


# ---------------------------------------------------------------- device kernel

def _build_program():
    _ensure_paths()
    import concourse.bass as bass
    import concourse.mybir as mybir
    import concourse.tile as tile
    from concourse import bacc
    from concourse.bass import ds, ts

    f32 = mybir.dt.float32
    fmm = {"f32r": mybir.dt.float32r,
           "f16": mybir.dt.float16}.get(MM_DT, f32)
    AF = mybir.ActivationFunctionType
    OP = mybir.AluOpType

    nc = bacc.Bacc("TRN2", target_bir_lowering=False, debug=False,
                   num_devices=NCORES)
    xT = nc.dram_tensor("xT", [IN, NG * T], fmm, kind="ExternalInput").ap()
    wT = nc.dram_tensor("wT", [E, IN, OS], f32, kind="ExternalInput").ap()
    rT = nc.dram_tensor("rT", [IN, OS], f32, kind="ExternalInput").ap()
    sgT = nc.dram_tensor("sgT", [IN, OS], f32, kind="ExternalInput").ap()
    Bf = nc.dram_tensor("Bf", [P, 2, IN], fmm, kind="ExternalInput").ap()
    Ah = nc.dram_tensor("Ah", [P, 2, OS], fmm, kind="ExternalInput").ap()
    gm = nc.dram_tensor("gm", [P, NG], fmm, kind="ExternalInput").ap()
    rTr = nc.dram_tensor("rTr", [IN, OS], fmm, kind="ExternalInput").ap()
    bT = nc.dram_tensor("bT", [OS, NG], f32, kind="ExternalInput").ap()
    yO = nc.dram_tensor("y", [NG, OS, T], f32, kind="ExternalOutput").ap()
    Dscr = nc.dram_tensor("Dscr", [IT // 2, P, E + 1, 2, OS], fmm).ap()
    Wscr = nc.dram_tensor("Wscr", [NG, IN, OS], fmm).ap()

    QIT = 2                       # k-tiles per quarter
    NQ = IT // QIT                # 4 quarters

    with tile.TileContext(nc) as tc:
        from contextlib import ExitStack
        with ExitStack() as ctx:
            const = ctx.enter_context(tc.tile_pool(name="const", bufs=1))
            wpool = ctx.enter_context(tc.tile_pool(name="wpool", bufs=2))
            dpool = ctx.enter_context(tc.tile_pool(name="dpool", bufs=2))
            Wp = ctx.enter_context(tc.tile_pool(name="Wp", bufs=5))
            ypool = ctx.enter_context(tc.tile_pool(name="ypool", bufs=1))
            mp = ctx.enter_context(tc.tile_pool(name="mp", bufs=4))
            rq = ctx.enter_context(tc.tile_pool(name="rq", bufs=1))
            sp = ctx.enter_context(tc.tile_pool(name="sp", bufs=3))
            cb = ctx.enter_context(tc.tile_pool(name="cb", bufs=3))
            xp = ctx.enter_context(tc.tile_pool(name="xp", bufs=8))
            pr_p = ctx.enter_context(tc.tile_pool(name="pr", bufs=2, space="PSUM"))
            pc_p = ctx.enter_context(tc.tile_pool(name="pc", bufs=3, space="PSUM"))
            py_p = ctx.enter_context(tc.tile_pool(name="py", bufs=3, space="PSUM"))

            B_sb = const.tile([P, 2, IN], fmm)
            nc.sync.dma_start(B_sb[:], Bf)
            A_sb = const.tile([P, 2, OS], fmm)
            nc.sync.dma_start(A_sb[:], Ah)
            g_sb = const.tile([P, NG], fmm)
            nc.sync.dma_start(g_sb[:], gm)
            b_sb = const.tile([OS, NG], f32)
            nc.sync.dma_start(b_sb[:], bT)
            r_sb = const.tile([P, IT, OS], f32)
            nc.sync.dma_start(r_sb[:], rT.rearrange("(it p) o -> p it o", p=P))
            s_sb = const.tile([P, IT, OS], f32)
            nc.sync.dma_start(s_sb[:], sgT.rearrange("(it p) o -> p it o", p=P))

            yac = ypool.tile([OS, NG, T], f32)

            for qh in range(NQ):
                it0 = qh * QIT

                # ---- per-quarter weight load + masks + rank-8 delta
                # Dt [ip, e9, jt, o]; row e9==E holds the residual weights.
                wt = wpool.tile([P, E, QIT, OS], f32, tag="wt")
                for e in range(E):
                    nc.sync.dma_start(
                        wt[:, e],
                        wT[e, ds(it0 * P, QIT * P), :].rearrange(
                            "(it p) o -> p it o", p=P))
                Dt = dpool.tile([P, E + 1, QIT, OS], fmm, tag="Dt")
                nc.scalar.dma_start(
                    Dt[:, E],
                    rTr[ds(it0 * P, QIT * P), :].rearrange(
                        "(it p) o -> p it o", p=P))
                for e in range(E):
                    d = mp.tile([P, QIT, OS], f32, tag="t1")
                    nc.gpsimd.tensor_tensor(d[:], wt[:, e],
                                            r_sb[:, ds(it0, QIT)],
                                            OP.subtract)
                    u2 = mp.tile([P, QIT, OS], f32, tag="t2")
                    nc.gpsimd.tensor_tensor(u2[:], d[:],
                                            s_sb[:, ds(it0, QIT)], OP.mult)
                    m = mp.tile([P, QIT, OS], f32, tag="t5")
                    nc.vector.tensor_scalar(m[:], u2[:], 0.0, None, OP.is_gt)
                    msg = mp.tile([P, QIT, OS], f32, tag="t3")
                    nc.vector.tensor_tensor(msg[:], m[:],
                                            s_sb[:, ds(it0, QIT)], OP.mult)
                    for j in range(QIT):
                        it = it0 + j
                        prt = pr_p.tile([P, OS], f32, tag="pr")
                        nc.tensor.matmul(prt[:],
                                         B_sb[ds(32 * (e % 4), RANK),
                                              e // 4, ts(it, P)],
                                         A_sb[ds(32 * (e % 4), RANK),
                                              e // 4, :],
                                         start=True, stop=True,
                                         tile_position=(32 * (e % 4), 0))
                        nc.vector.tensor_tensor(Dt[:, e, j], msg[:, j],
                                                prt[:], OP.mult)

                # ---- combine + drain into W quarter
                # Partition p' of Dt holds i_loc = 16*(p'%32//4) + 4*(p'//32)
                # + p'%4 (host-permuted), so the e-partition restaging and
                # the W drain are each a handful of big <=3-dim DMAs through
                # DRAM scratch. MM q = 4*G+rg covers rhs slots 4G..4G+3 of
                # row-block rg (slot sl = p' - 32*rg).
                nc.gpsimd.dma_start(Dscr[qh], Dt[:])
                dma_rot = [nc.sync, nc.scalar, nc.gpsimd]
                Whs = []
                for j in range(QIT):
                    rhs = sp.tile([P, 32, OS], fmm, tag="rhs")
                    for rg in range(4):
                        nc.sync.dma_start(
                            rhs[ds(32 * rg, E + 1), :, :],
                            bass.AP(tensor=Dscr.tensor,
                                    offset=qh * (P * (E + 1) * QIT * OS)
                                    + 32 * rg * ((E + 1) * QIT * OS)
                                    + j * OS,
                                    ap=[[QIT * OS, E + 1],
                                        [(E + 1) * QIT * OS, 32],
                                        [1, OS]]))
                    for G in range(8):
                        ps4 = pc_p.tile([P, 4, OS], f32, tag="pc")
                        for rg in range(4):
                            nc.tensor.matmul(
                                ps4[ds(32 * rg, NG), :, :],
                                g_sb[ds(32 * rg, E + 1), :],
                                rhs[ds(32 * rg, E + 1), ds(4 * G, 4), :],
                                start=True, stop=True,
                                tile_position=(32 * rg, 32 * rg))
                        cbuf = cb.tile([P, 4, OS], fmm, tag="cb")
                        if G % 2 == 0:
                            nc.vector.tensor_copy(cbuf[:], ps4[:])
                        else:
                            nc.scalar.activation(cbuf[:], ps4[:], AF.Copy)
                        dma_rot[G % 3].dma_start(
                            bass.AP(tensor=Wscr.tensor,
                                    offset=((it0 + j) * P + 4 * G) * OS,
                                    ap=[[32 * OS, 4], [IN * OS, NG],
                                        [1, 4 * OS]]),
                            cbuf[:])
                    # ---- W readback for this j
                    Wh = Wp.tile([P, NG, OS], fmm, tag="W")
                    nc.scalar.dma_start(
                        Wh[:],
                        Wscr[:, ds((it0 + j) * P, P), :].rearrange(
                            "g i o -> i g o"))
                    Whs.append(Wh)

                # ---- main matmul for this quarter (chained over j in PSUM)
                for gp in range(NG // 2):
                    xts = []
                    for j in range(QIT):
                        it = it0 + j
                        xt = xp.tile([P, 2 * T], fmm, tag="xt")
                        eng = nc.sync if (gp + j) % 2 == 0 else nc.scalar
                        eng.dma_start(xt[:], xT[ds(it * P, P),
                                                ds(gp * 2 * T, 2 * T)])
                        xts.append(xt)
                    for gl in range(2):
                        g = 2 * gp + gl
                        pyt = py_p.tile([P, T], f32, tag="py")
                        for j in range(QIT):
                            nc.tensor.matmul(pyt[:], Whs[j][:, g, :],
                                             xts[j][:, ds(gl * T, T)],
                                             start=(j == 0),
                                             stop=(j == QIT - 1))
                        if qh == 0:
                            nc.scalar.activation(yac[:, g, :], pyt[:],
                                                 AF.Identity,
                                                 bias=b_sb[:, ds(g, 1)])
                        else:
                            nc.vector.tensor_tensor(yac[:, g, :],
                                                    yac[:, g, :],
                                                    pyt[:], OP.add)

            nc.sync.dma_start(yO.rearrange("g o t -> o g t"), yac[:])

    nc.compile()
    return nc


def _get_program():
    global _PROG
    if _PROG is None:
        _PROG = _build_program()
    return _PROG


# ---------------------------------------------------------------- entry point

def _run(in_maps, trace=False):
    _ensure_paths()
    from concourse.bass_utils import run_bass_kernel_spmd
    nc = _get_program()
    return run_bass_kernel_spmd(nc, in_maps, core_ids=list(range(NCORES)),
                                trace=trace)


def kernel(**inputs):
    _ensure_paths()
    in_maps, loss = _host_prep(**inputs)
    res = _run(in_maps, trace=False)
    ys = [res.results[c]["y"] for c in range(NCORES)]
    Y = np.stack(ys)                                   # [C, NG, OS, T]
    y = np.transpose(Y, (1, 3, 0, 2)).reshape(NG, T, OUT)
    y = np.ascontiguousarray(y.reshape(B_DIM, L, OUT), dtype=np.float32)
    return y, loss


# revision 31
# speedup vs baseline: 1.1029x; 1.0176x over previous
"""Trainium2 Bass kernel for nn_Conv1D_MEO (MoE ties-merging + SVD/Kron low-rank).

Strategy (8 NeuronCores, output-channel sharded):
  * Host: SVD top-8 left singular vectors (jax-CPU, matches reference LAPACK
    signs), Kron curve factors -> per-expert rank-8 factors A,B; gating /
    softmax / loss / bias path (all tiny).
  * Device, core c owns OUT rows [128c, 128c+128):
      - ties-merging sign masks (elementwise, DVE+ACT),
      - rank-8 delta reconstruction R = B^T A^T (PE),
      - per-token-group weight combine W_g = res + sum_e g_ge*(mask_e*R_e)
        as K=9 PE matmuls, diagonally tile_position-packed 4x,
      - main matmul y_g^T = W_g^T-slice @ x_g^T (PE, K=1024 accumulation).
  * Host gathers [o,t]-transposed per-core outputs into [B, L, OUT].
"""

import numpy as np

E = 8
IN = 1024
OUT = 1024
B_DIM = 4
L = 2048
T = 256
N = L // T            # 8 groups per batch
NG = B_DIM * N        # 32 token groups
RANK = 8
NCORES = 8
OS = OUT // NCORES    # 128 out channels per core
P = 128
IT = IN // P          # 8 k-tiles
HIT = IT // 2         # k-tiles per half
LOSS_COEF = 1e-3

_PROG = None  # cached (nc, input names)
MM_DT = "f16"   # matmul data dtype: "f16" | "f32r" | "f32" (masks/accum always f32)


def _ensure_paths():
    import sys
    for p in ("/opt/trn_rl_repo", "/opt/pypackages"):
        if p not in sys.path:
            sys.path.append(p)


# ---------------------------------------------------------------- host math

def _softmax_f32(x):
    m = x.max(axis=1, keepdims=True)
    e = np.exp((x - m).astype(np.float32))
    return (e / e.sum(axis=1, keepdims=True)).astype(np.float32)


def _cv_squared(v):
    v = np.asarray(v, np.float32)
    if v.shape[0] == 1:
        return np.float32(0.0)
    return np.float32(np.var(v, ddof=1) / (np.mean(v) ** 2 + 1e-10))


def _host_prep(x, w_gate, weight, bias, res_weight, res_bias,
               curve1_in, curve2_in, curve1_out, curve2_out,
               curve1_bias, curve2_bias):
    """All small/host-side math. Returns per-core input maps + loss."""
    import jax
    import jax.numpy as jnp
    cpu = jax.devices("cpu")[0]

    f32 = np.float32
    x = np.asarray(x, f32)
    diff_w = (np.asarray(weight, f32) - np.asarray(res_weight, f32)[None])

    # --- SVD on jax-CPU: must match the reference's LAPACK sign conventions.
    with jax.default_device(cpu):
        U = np.asarray(jnp.linalg.svd(jnp.asarray(diff_w),
                                      full_matrices=False)[0][:, :, :RANK])
        # sign of the expert-summed weight delta, computed with the exact
        # same reduction the reference uses (fp32 sum order matters on
        # near-zero elements).
        sg_full = np.asarray(jnp.sign(jnp.sum(jnp.asarray(diff_w), axis=0)))
        diff_b = (np.asarray(bias, f32) - np.asarray(res_bias, f32))
        sb = np.asarray(jnp.sign(jnp.asarray(diff_b)))
        ssb = np.asarray(jnp.sign(jnp.sum(jnp.asarray(diff_b), axis=0)))

    # --- low-rank + Kron-factorized reconstruction (tiny matrices)
    Pm = np.swapaxes(U, 1, 2)                       # [E, r, OUT]
    rtw = np.matmul(Pm, diff_w)                     # [E, r, IN]
    rtw = rtw.reshape(E, 2, 4, 32, 32)
    c1o = np.asarray(curve1_out, f32)
    c2o = np.asarray(curve2_out, f32)
    c1i = np.asarray(curve1_in, f32)
    c2i = np.asarray(curve2_in, f32)
    rtw = np.einsum("bij,bjklm->biklm", c1o, rtw).astype(f32)
    rtw = np.einsum("bik,bjklm->bjilm", c2o, rtw).astype(f32)
    rtw = np.einsum("bil,bjklm->bjkim", c1i, rtw).astype(f32)
    rtw = np.einsum("bim,bjklm->bjkli", c2i, rtw).astype(f32)
    Bfac = rtw.reshape(E, RANK, IN)                 # [E, r, IN]
    Ahalf = U.astype(f32)                           # [E, OUT, r]

    # --- gating
    xg = x.reshape(NG, T, IN)
    mx = xg.mean(axis=1, dtype=f32)                 # [NG, IN]
    logits = (mx @ np.asarray(w_gate, f32)).astype(f32)
    gates = _softmax_f32(logits)                    # [NG, E]
    importance = gates.sum(axis=0, dtype=f32)
    load = (gates > 0).sum(axis=0).astype(f32)
    loss = np.float32((_cv_squared(importance) + _cv_squared(load)) * LOSS_COEF)

    e_first = gates.reshape(B_DIM, N, E)[:, 0].copy()
    g2 = np.roll(gates, 1, axis=0).reshape(B_DIM, N, E).copy()
    g2[:, 0] = e_first
    gates_f = np.ascontiguousarray(g2.reshape(NG, E))

    # --- bias path (all tiny)
    mask_b = (sb * (sb == ssb)).astype(f32)
    c1b = np.asarray(curve1_bias, f32)
    c2b = np.asarray(curve2_bias, f32)
    rtb = diff_b.reshape(E, 32, 32)
    rtb = np.einsum("bki,bij->bkj", c1b, rtb).astype(f32)
    rtb = np.einsum("bkj,bij->bik", c2b, rtb).astype(f32)
    rtb = rtb.reshape(E, OUT)
    expert_bias = (np.asarray(res_bias, f32)
                   + gates_f @ (rtb * mask_b)).astype(f32)   # [NG, OUT]

    # --- gate matrix for the device combine: 4 row-group replicas of
    # [gates^T ; ones] padded to 32 rows each.
    gmat = np.zeros((P, NG), f32)
    for rg in range(4):
        gmat[32 * rg:32 * rg + E] = gates_f.T
        gmat[32 * rg + E] = 1.0

    # --- per-core device inputs
    mmnp = {"f16": np.float16}.get(MM_DT, np.float32)
    # B/A rank factors packed across 128 partitions: expert e sits at
    # partitions 32*(e%4)+r, free slot e//4  (row-group packing for the PE).
    Bpack = np.zeros((P, 2, IN), f32)
    Apack = np.zeros((P, 2, OUT), f32)
    for e in range(E):
        Bpack[32 * (e % 4):32 * (e % 4) + RANK, e // 4] = Bfac[e]
        Apack[32 * (e % 4):32 * (e % 4) + RANK, e // 4] = \
            Ahalf[e].T                                       # [r, OUT]
    # Partition permutation: device partition p' holds original i-row
    # inv_perm[p'] within each 128-block, chosen so that the DRAM-scratch
    # restaging/drain DMAs have <=3 strided dims.
    ip = np.array([16 * ((pp % 32) // 4) + 4 * (pp // 32) + pp % 4
                   for pp in range(P)])
    rowperm = np.concatenate([blk * P + ip for blk in range(IN // P)])

    xT = np.ascontiguousarray(x.reshape(NG * T, IN).T[rowperm].astype(mmnp))
    in_maps = []
    for c in range(NCORES):
        osl = slice(OS * c, OS * (c + 1))
        in_maps.append({
            "xT": xT,
            "wT": np.ascontiguousarray(
                np.asarray(weight, f32)[:, osl, :].transpose(0, 2, 1)[:, rowperm]),
            "rT": np.ascontiguousarray(
                np.asarray(res_weight, f32)[osl, :].T[rowperm]),
            "sgT": np.ascontiguousarray(sg_full[osl, :].T[rowperm].astype(f32)),
            "Bf": np.ascontiguousarray(Bpack[:, :, rowperm].astype(mmnp)),
            "Ah": np.ascontiguousarray(Apack[:, :, osl].astype(mmnp)),
            "gm": gmat.astype(mmnp),
            "rTr": np.ascontiguousarray(
                np.asarray(res_weight, f32)[osl, :].T[rowperm].astype(mmnp)),
            "bT": np.ascontiguousarray(expert_bias[:, osl].T),       # [OS, NG]
        })
    return in_maps, loss


# ---------------------------------------------------------------- device kernel

def _build_program():
    _ensure_paths()
    import concourse.bass as bass
    import concourse.mybir as mybir
    import concourse.tile as tile
    from concourse import bacc
    from concourse.bass import ds, ts

    f32 = mybir.dt.float32
    fmm = {"f32r": mybir.dt.float32r,
           "f16": mybir.dt.float16}.get(MM_DT, f32)
    AF = mybir.ActivationFunctionType
    OP = mybir.AluOpType

    nc = bacc.Bacc("TRN2", target_bir_lowering=False, debug=False,
                   num_devices=NCORES)
    xT = nc.dram_tensor("xT", [IN, NG * T], fmm, kind="ExternalInput").ap()
    wT = nc.dram_tensor("wT", [E, IN, OS], f32, kind="ExternalInput").ap()
    rT = nc.dram_tensor("rT", [IN, OS], f32, kind="ExternalInput").ap()
    sgT = nc.dram_tensor("sgT", [IN, OS], f32, kind="ExternalInput").ap()
    Bf = nc.dram_tensor("Bf", [P, 2, IN], fmm, kind="ExternalInput").ap()
    Ah = nc.dram_tensor("Ah", [P, 2, OS], fmm, kind="ExternalInput").ap()
    gm = nc.dram_tensor("gm", [P, NG], fmm, kind="ExternalInput").ap()
    rTr = nc.dram_tensor("rTr", [IN, OS], fmm, kind="ExternalInput").ap()
    bT = nc.dram_tensor("bT", [OS, NG], f32, kind="ExternalInput").ap()
    yO = nc.dram_tensor("y", [NG, OS, T], f32, kind="ExternalOutput").ap()
    Dscr = nc.dram_tensor("Dscr", [IT // 2, P, E + 1, 2, OS], fmm).ap()
    Wscr = nc.dram_tensor("Wscr", [NG, IN, OS], fmm).ap()

    QIT = 2                       # k-tiles per quarter
    NQ = IT // QIT                # 4 quarters

    with tile.TileContext(nc) as tc:
        from contextlib import ExitStack
        with ExitStack() as ctx:
            const = ctx.enter_context(tc.tile_pool(name="const", bufs=1))
            wpool = ctx.enter_context(tc.tile_pool(name="wpool", bufs=2))
            dpool = ctx.enter_context(tc.tile_pool(name="dpool", bufs=2))
            Wp = ctx.enter_context(tc.tile_pool(name="Wp", bufs=5))
            ypool = ctx.enter_context(tc.tile_pool(name="ypool", bufs=1))
            mp = ctx.enter_context(tc.tile_pool(name="mp", bufs=4))
            rq = ctx.enter_context(tc.tile_pool(name="rq", bufs=1))
            sp = ctx.enter_context(tc.tile_pool(name="sp", bufs=3))
            cb = ctx.enter_context(tc.tile_pool(name="cb", bufs=3))
            xp = ctx.enter_context(tc.tile_pool(name="xp", bufs=8))
            pr_p = ctx.enter_context(tc.tile_pool(name="pr", bufs=2, space="PSUM"))
            pc_p = ctx.enter_context(tc.tile_pool(name="pc", bufs=3, space="PSUM"))
            py_p = ctx.enter_context(tc.tile_pool(name="py", bufs=3, space="PSUM"))

            B_sb = const.tile([P, 2, IN], fmm)
            nc.sync.dma_start(B_sb[:], Bf)
            A_sb = const.tile([P, 2, OS], fmm)
            nc.sync.dma_start(A_sb[:], Ah)
            g_sb = const.tile([P, NG], fmm)
            nc.sync.dma_start(g_sb[:], gm)
            b_sb = const.tile([OS, NG], f32)
            nc.sync.dma_start(b_sb[:], bT)
            r_sb = const.tile([P, IT, OS], f32)
            nc.sync.dma_start(r_sb[:], rT.rearrange("(it p) o -> p it o", p=P))
            s_sb = const.tile([P, IT, OS], f32)
            nc.sync.dma_start(s_sb[:], sgT.rearrange("(it p) o -> p it o", p=P))

            yac = ypool.tile([OS, NG, T], f32)

            def phase_mask(qh):
                it0 = qh * QIT
                # Dt [ip, e9, jt, o]; row e9==E holds the residual weights.
                wt = wpool.tile([P, E, QIT, OS], f32, tag="wt")
                for e in range(E):
                    nc.sync.dma_start(
                        wt[:, e],
                        wT[e, ds(it0 * P, QIT * P), :].rearrange(
                            "(it p) o -> p it o", p=P))
                Dt = dpool.tile([P, E + 1, QIT, OS], fmm, tag="Dt")
                nc.scalar.dma_start(
                    Dt[:, E],
                    rTr[ds(it0 * P, QIT * P), :].rearrange(
                        "(it p) o -> p it o", p=P))
                for e in range(E):
                    d = mp.tile([P, QIT, OS], f32, tag="t1")
                    nc.gpsimd.tensor_tensor(d[:], wt[:, e],
                                            r_sb[:, ds(it0, QIT)],
                                            OP.subtract)
                    u2 = mp.tile([P, QIT, OS], f32, tag="t2")
                    nc.gpsimd.tensor_tensor(u2[:], d[:],
                                            s_sb[:, ds(it0, QIT)], OP.mult)
                    m = mp.tile([P, QIT, OS], f32, tag="t5")
                    nc.vector.tensor_scalar(m[:], u2[:], 0.0, None, OP.is_gt)
                    msg = mp.tile([P, QIT, OS], f32, tag="t3")
                    nc.vector.tensor_tensor(msg[:], m[:],
                                            s_sb[:, ds(it0, QIT)], OP.mult)
                    for j in range(QIT):
                        it = it0 + j
                        prt = pr_p.tile([P, OS], f32, tag="pr")
                        nc.tensor.matmul(prt[:],
                                         B_sb[ds(32 * (e % 4), RANK),
                                              e // 4, ts(it, P)],
                                         A_sb[ds(32 * (e % 4), RANK),
                                              e // 4, :],
                                         start=True, stop=True,
                                         tile_position=(32 * (e % 4), 0))
                        nc.vector.tensor_tensor(Dt[:, e, j], msg[:, j],
                                                prt[:], OP.mult)
                nc.gpsimd.dma_start(Dscr[qh], Dt[:])

            def phase_combine(qh):
                # Partition p' of Dt holds i_loc = 16*(p'%32//4) + 4*(p'//32)
                # + p'%4 (host-permuted), so the e-partition restaging and
                # the W drain are each a handful of big <=3-dim DMAs through
                # DRAM scratch. MM q = 4*G+rg covers rhs slots 4G..4G+3 of
                # row-block rg (slot sl = p' - 32*rg).
                it0 = qh * QIT
                dma_rot = [nc.sync, nc.scalar, nc.gpsimd]
                Whs = []
                for j in range(QIT):
                    rhs = sp.tile([P, 32, OS], fmm, tag="rhs")
                    for rg in range(4):
                        nc.sync.dma_start(
                            rhs[ds(32 * rg, E + 1), :, :],
                            bass.AP(tensor=Dscr.tensor,
                                    offset=qh * (P * (E + 1) * QIT * OS)
                                    + 32 * rg * ((E + 1) * QIT * OS)
                                    + j * OS,
                                    ap=[[QIT * OS, E + 1],
                                        [(E + 1) * QIT * OS, 32],
                                        [1, OS]]))
                    for G in range(8):
                        ps4 = pc_p.tile([P, 4, OS], f32, tag="pc")
                        for rg in range(4):
                            nc.tensor.matmul(
                                ps4[ds(32 * rg, NG), :, :],
                                g_sb[ds(32 * rg, E + 1), :],
                                rhs[ds(32 * rg, E + 1), ds(4 * G, 4), :],
                                start=True, stop=True,
                                tile_position=(32 * rg, 32 * rg))
                        cbuf = cb.tile([P, 4, OS], fmm, tag="cb")
                        if G % 2 == 0:
                            nc.vector.tensor_copy(cbuf[:], ps4[:])
                        else:
                            nc.scalar.activation(cbuf[:], ps4[:], AF.Copy)
                        dma_rot[G % 3].dma_start(
                            bass.AP(tensor=Wscr.tensor,
                                    offset=((it0 + j) * P + 4 * G) * OS,
                                    ap=[[32 * OS, 4], [IN * OS, NG],
                                        [1, 4 * OS]]),
                            cbuf[:])
                    Wh = Wp.tile([P, NG, OS], fmm, tag="W")
                    nc.scalar.dma_start(
                        Wh[:],
                        Wscr[:, ds((it0 + j) * P, P), :].rearrange(
                            "g i o -> i g o"))
                    Whs.append(Wh)
                return Whs

            def phase_main(qh, Whs):
                it0 = qh * QIT
                for gp in range(NG // 2):
                    xts = []
                    for j in range(QIT):
                        it = it0 + j
                        xt = xp.tile([P, 2 * T], fmm, tag="xt")
                        eng = nc.sync if (gp + j) % 2 == 0 else nc.scalar
                        eng.dma_start(xt[:], xT[ds(it * P, P),
                                                ds(gp * 2 * T, 2 * T)])
                        xts.append(xt)
                    for gl in range(2):
                        g = 2 * gp + gl
                        pyt = py_p.tile([P, T], f32, tag="py")
                        for j in range(QIT):
                            nc.tensor.matmul(pyt[:], Whs[j][:, g, :],
                                             xts[j][:, ds(gl * T, T)],
                                             start=(j == 0),
                                             stop=(j == QIT - 1))
                        if qh == 0:
                            nc.scalar.activation(yac[:, g, :], pyt[:],
                                                 AF.Identity,
                                                 bias=b_sb[:, ds(g, 1)])
                        else:
                            nc.vector.tensor_tensor(yac[:, g, :],
                                                    yac[:, g, :],
                                                    pyt[:], OP.add)

            # software pipeline: masks(q+1) is emitted before main(q) so the
            # in-order engine streams never block next-quarter mask work
            # behind current-quarter drain waits.
            phase_mask(0)
            W0 = phase_combine(0)
            Whs_prev = W0
            for q in range(1, NQ):
                phase_mask(q)
                phase_main(q - 1, Whs_prev)
                Whs_prev = phase_combine(q)
            phase_main(NQ - 1, Whs_prev)

            nc.sync.dma_start(yO.rearrange("g o t -> o g t"), yac[:])

    nc.compile()
    return nc


def _get_program():
    global _PROG
    if _PROG is None:
        _PROG = _build_program()
    return _PROG


# ---------------------------------------------------------------- entry point

def _run(in_maps, trace=False):
    _ensure_paths()
    from concourse.bass_utils import run_bass_kernel_spmd
    nc = _get_program()
    return run_bass_kernel_spmd(nc, in_maps, core_ids=list(range(NCORES)),
                                trace=trace)


def kernel(**inputs):
    _ensure_paths()
    in_maps, loss = _host_prep(**inputs)
    res = _run(in_maps, trace=False)
    ys = [res.results[c]["y"] for c in range(NCORES)]
    Y = np.stack(ys)                                   # [C, NG, OS, T]
    y = np.transpose(Y, (1, 3, 0, 2)).reshape(NG, T, OUT)
    y = np.ascontiguousarray(y.reshape(B_DIM, L, OUT), dtype=np.float32)
    return y, loss
